# Initial kernel scaffold
#
"""Sparse (range-masked) GQA attention on 8 Trainium2 NeuronCores.

Strategy: tensor-parallel over heads. Core c owns q heads {2c, 2c+1} and kv
head c//2, so each core runs an independent single-core attention over the
full sequence — no collectives, and the host concatenates the 8 head-slices.

Per-core kernel (Bass/Tile):
  1. DMA q/k/v/cos/sin shards into SBUF.
  2. RoPE in fp32 on the vector engine, writing bf16.
  3. PE-transpose roped q/k into QT/KT strips laid out [d=128, T].
  4. For each (head, 512-wide q segment, active 128-wide k chunk):
       S^T[tk,tq] = KT_chunk.T @ QT_seg        (PE, bf16, PSUM fp32)
       P^T = Exp(scale*S^T - 4)                (ACT, PSUM -> SBUF bf16)
       partial tiles only: P^T *= mask01       (mask built by affine_select)
       out[tq,0:129] += P^T_block.T @ [V | 1]  (PE, ones column => row sums)
  5. out = PV / max(rowsum, tiny); rows with no coverage come out as 0.

The tile schedule (skip / dense / partial+rectangles) is computed on the host
from q_ranges/k_ranges — scheduling metadata only; all arithmetic on q/k/v
runs on device. The compiled NEFF is cached per schedule signature.
"""

import math

import numpy as np

T, HQ, HKV, D = 2048, 16, 4, 128
N_CORES = 8
HPC = HQ // N_CORES          # q heads per core
SEG = 512                    # tq segment width (max moving free dim)
CK = 128                     # tk chunk (contraction tile)
NSEG = T // SEG
NCK = T // CK
NT = T // 128                # 128-row t-tiles
HALF = D // 2
SCALE = 1.0 / math.sqrt(D)
EXP_BIAS = -4.0              # constant shift; cancels in softmax normalization
SUM_EPS = 1e-30

PROFILE = False              # set True (e.g. from test.py) to capture a trace
LAST_EXEC_NS = None
LAST_RESULT = None

_NEFF_CACHE = {}


def _build_schedule(q_ranges, k_ranges):
    """Per (qseg, kchunk) tile: absent=skip, None=dense, list=mask rectangles.

    Rectangles are in device-tile coordinates: (klo, khi) along the partition
    (tk) axis, (qlo, qhi) along the free (tq) axis, each clipped to the tile.
    """
    q_ranges = np.asarray(q_ranges, dtype=np.int64)
    k_ranges = np.asarray(k_ranges, dtype=np.int64)
    idx = np.arange(T)
    qm = (idx[None, :] >= q_ranges[:, :1]) & (idx[None, :] < q_ranges[:, 1:])
    km = (idx[None, :] >= k_ranges[:, :1]) & (idx[None, :] < k_ranges[:, 1:])
    # q-block r covered columns & k coverage; mask = union over r of rects
    mask = np.zeros((T, T), dtype=bool)
    for r in range(len(q_ranges)):
        if qm[r].any() and km[r].any():
            mask[np.ix_(qm[r], km[r])] = True
    sched = {}
    for s in range(NSEG):
        for c in range(NCK):
            sub = mask[s * SEG:(s + 1) * SEG, c * CK:(c + 1) * CK]
            if not sub.any():
                continue
            if sub.all():
                sched[(s, c)] = None
                continue
            rects = []
            for r in range(len(q_ranges)):
                qlo = max(int(q_ranges[r, 0]) - s * SEG, 0)
                qhi = min(int(q_ranges[r, 1]) - s * SEG, SEG)
                klo = max(int(k_ranges[r, 0]) - c * CK, 0)
                khi = min(int(k_ranges[r, 1]) - c * CK, CK)
                if qlo < qhi and klo < khi:
                    rects.append((klo, khi, qlo, qhi))
            sched[(s, c)] = rects
    return sched


def _sched_key(sched):
    return tuple(sorted(
        (k, -1) if v is None else (k, tuple(v)) for k, v in sched.items()
    ))


def _build_nc(sched):
    from contextlib import ExitStack

    from concourse import bacc, bass, mybir, tile

    f32 = mybir.dt.float32
    bf16 = mybir.dt.bfloat16
    AO = mybir.AluOpType
    AF = mybir.ActivationFunctionType

    nc = bacc.Bacc(None, target_bir_lowering=False)
    q_ext = nc.declare_dram_parameter("q", [T, HPC, D], f32, isOutput=False)
    k_ext = nc.declare_dram_parameter("k", [T, 1, D], f32, isOutput=False)
    v_ext = nc.declare_dram_parameter("v", [T, 1, D], f32, isOutput=False)
    cos_ext = nc.declare_dram_parameter("cos", [T, HALF], f32, isOutput=False)
    sin_ext = nc.declare_dram_parameter("sin", [T, HALF], f32, isOutput=False)
    out_ext = nc.declare_dram_parameter("out", [T, HPC, D], f32, isOutput=True)

    with tile.TileContext(nc) as tc, ExitStack() as ctx:
        const = ctx.enter_context(tc.tile_pool(name="const", bufs=1))
        ps_pool = ctx.enter_context(
            tc.tile_pool(name="psum", bufs=2, space="PSUM"))
        pt_sb_pool = ctx.enter_context(tc.tile_pool(name="ptsb", bufs=3))
        m01_pool = ctx.enter_context(tc.tile_pool(name="m01", bufs=2))
        out_pool = ctx.enter_context(tc.tile_pool(name="outp", bufs=4))
        stat_pool = ctx.enter_context(tc.tile_pool(name="stat", bufs=8))

        # ---------------- loads ----------------
        q_sb = const.tile([128, NT, HPC, D], f32, tag="q_sb")
        nc.sync.dma_start(
            q_sb[:], q_ext[:, :, :].rearrange("(n p) h d -> p n h d", p=128))
        k_sb = const.tile([128, NT, D], f32, tag="k_sb")
        nc.sync.dma_start(
            k_sb[:], k_ext[:, :, :].rearrange("(n p) h d -> p n (h d)", p=128))
        v_sb = const.tile([128, NT, D], f32, tag="v_sb")
        nc.sync.dma_start(
            v_sb[:], v_ext[:, :, :].rearrange("(n p) h d -> p n (h d)", p=128))
        cos_sb = const.tile([128, NT, HALF], f32, tag="cos_sb")
        nc.sync.dma_start(
            cos_sb[:], cos_ext[:, :].rearrange("(n p) d -> p n d", p=128))
        sin_sb = const.tile([128, NT, HALF], f32, tag="sin_sb")
        nc.sync.dma_start(
            sin_sb[:], sin_ext[:, :].rearrange("(n p) d -> p n d", p=128))

        # identity for PE transposes
        ident = const.tile([128, 128], bf16, tag="ident")
        nc.gpsimd.memset(ident[:], 0.0)
        nc.gpsimd.affine_select(
            out=ident[:], in_=ident[:], compare_op=AO.not_equal, fill=1.0,
            base=0, pattern=[[-1, 128]], channel_multiplier=1)

        # V with an appended ones column: [tk, (V | 1)] in bf16
        vaug = const.tile([128, NT, D + 1], bf16, tag="vaug")
        nc.vector.memset(vaug[:], 1.0)
        nc.vector.tensor_copy(vaug[:, :, 0:D], v_sb[:])

        # ---------------- rope (fp32 -> bf16) ----------------
        t1 = const.tile([128, NT, HALF], f32, tag="t1")
        t2 = const.tile([128, NT, HALF], f32, tag="t2")
        roped = []  # bf16 [128, NT, D] per (q head 0, q head 1, k)
        rope_srcs = [q_sb[:, :, h, :] for h in range(HPC)] + [k_sb[:]]
        for i, x in enumerate(rope_srcs):
            dst = const.tile([128, NT, D], bf16, tag=f"rope{i}")
            xlo, xhi = x[:, :, 0:HALF], x[:, :, HALF:D]
            nc.vector.tensor_tensor(t1[:], xhi, sin_sb[:], AO.mult)
            nc.vector.tensor_tensor(t2[:], xlo, cos_sb[:], AO.mult)
            nc.vector.tensor_tensor(dst[:, :, 0:HALF], t2[:], t1[:], AO.subtract)
            nc.vector.tensor_tensor(t1[:], xhi, cos_sb[:], AO.mult)
            nc.vector.tensor_tensor(t2[:], xlo, sin_sb[:], AO.mult)
            nc.vector.tensor_tensor(dst[:, :, HALF:D], t1[:], t2[:], AO.add)
            roped.append(dst)

        # ---------------- transpose to [d, T] strips ----------------
        qt = [const.tile([128, T], bf16, tag=f"qt{h}") for h in range(HPC)]
        kt = const.tile([128, T], bf16, tag="kt")
        strips = [(roped[0], qt[0]), (roped[1], qt[1]), (roped[2], kt)]
        for n in range(NT):
            for src, strip in strips:
                ps_t = ps_pool.tile([128, 128], f32, tag="tp")
                nc.tensor.transpose(ps_t[:], src[:, n, :], ident[:])
                nc.vector.tensor_copy(strip[:, n * 128:(n + 1) * 128], ps_t[:])

        # zero tile for uncovered output rows
        zero_sb = const.tile([128, D], f32, tag="zero")
        nc.vector.memset(zero_sb[:], 0.0)

        # ---------------- main attention loop ----------------
        for h in range(HPC):
            for s in range(NSEG):
                chunks = [c for c in range(NCK) if (s, c) in sched]
                if not chunks:
                    for b in range(4):
                        q0 = s * SEG + b * 128
                        nc.sync.dma_start(
                            out_ext[q0:q0 + 128, h:h + 1, :], zero_sb[:])
                    continue
                pvA = ps_pool.tile([128, 3, D + 1], f32, tag="pvA")
                pvB = ps_pool.tile([128, D + 1], f32, tag="pvB")
                for i, c in enumerate(chunks):
                    ps_s = ps_pool.tile([128, SEG], f32, tag="ps")
                    nc.tensor.matmul(
                        ps_s[:], kt[:, c * CK:(c + 1) * CK],
                        qt[h][:, s * SEG:(s + 1) * SEG],
                        start=True, stop=True)
                    p_t = pt_sb_pool.tile([128, SEG], bf16, tag="pt")
                    nc.scalar.activation(
                        p_t[:], ps_s[:], AF.Exp, bias=EXP_BIAS, scale=SCALE)
                    rects = sched[(s, c)]
                    if rects is not None:
                        m01 = m01_pool.tile([128, SEG], bf16, tag="m")
                        if len(rects) == 1:
                            _rect_mask(nc, AO, m01[:], rects[0], first=True)
                        else:
                            acc = m01_pool.tile([128, SEG], bf16, tag="macc")
                            _rect_mask(nc, AO, m01[:], rects[0], first=True)
                            for rect in rects[1:]:
                                _rect_mask(nc, AO, acc[:], rect, first=True)
                                nc.vector.tensor_tensor(
                                    m01[:], m01[:], acc[:], AO.max)
                        nc.vector.tensor_tensor(p_t[:], p_t[:], m01[:], AO.mult)
                    first, last = i == 0, i == len(chunks) - 1
                    for b in range(4):
                        dst = pvA[:, b, :] if b < 3 else pvB[:]
                        nc.tensor.matmul(
                            dst, p_t[:, b * 128:(b + 1) * 128],
                            vaug[:, c, :], start=first, stop=last)
                for b in range(4):
                    src = pvA[:, b, :] if b < 3 else pvB[:]
                    sums = stat_pool.tile([128, 1], f32, tag="sums")
                    nc.vector.tensor_scalar_max(sums[:], src[:, D:D + 1], SUM_EPS)
                    rec = stat_pool.tile([128, 1], f32, tag="rec")
                    nc.vector.reciprocal(rec[:], sums[:])
                    o_sb = out_pool.tile([128, D], f32, tag="o")
                    nc.vector.tensor_scalar_mul(o_sb[:], src[:, 0:D], rec[:])
                    q0 = s * SEG + b * 128
                    nc.sync.dma_start(
                        out_ext[q0:q0 + 128, h:h + 1, :], o_sb[:])
    return nc


def _rect_mask(nc, AO, m, rect, first):
    """Build a {0,1} indicator of rect on a [CK, SEG] tile via affine_select.

    rect = (klo, khi, qlo, qhi) in tile-local coords; partition axis = tk,
    free axis = tq. Keep-predicates chain (AND); start from all-ones.
    """
    klo, khi, qlo, qhi = rect
    if first:
        nc.gpsimd.memset(m, 1.0)
    if klo > 0:
        nc.gpsimd.affine_select(
            out=m, in_=m, compare_op=AO.is_ge, fill=0.0,
            base=-klo, pattern=[[0, SEG]], channel_multiplier=1)
    if khi < CK:
        nc.gpsimd.affine_select(
            out=m, in_=m, compare_op=AO.is_lt, fill=0.0,
            base=-khi, pattern=[[0, SEG]], channel_multiplier=1)
    if qlo > 0:
        nc.gpsimd.affine_select(
            out=m, in_=m, compare_op=AO.is_ge, fill=0.0,
            base=-qlo, pattern=[[1, SEG]], channel_multiplier=0)
    if qhi < SEG:
        nc.gpsimd.affine_select(
            out=m, in_=m, compare_op=AO.is_lt, fill=0.0,
            base=-qhi, pattern=[[1, SEG]], channel_multiplier=0)


def _shards(q, k, v, cos, sin):
    in_maps = []
    for c in range(N_CORES):
        kv = c // 2
        in_maps.append({
            "q": np.ascontiguousarray(q[:, 2 * c:2 * c + 2, :], dtype=np.float32),
            "k": np.ascontiguousarray(k[:, kv:kv + 1, :], dtype=np.float32),
            "v": np.ascontiguousarray(v[:, kv:kv + 1, :], dtype=np.float32),
            "cos": np.ascontiguousarray(cos, dtype=np.float32),
            "sin": np.ascontiguousarray(sin, dtype=np.float32),
        })
    return in_maps


def kernel(q, k, v, cos, sin, q_ranges, k_ranges):
    global LAST_EXEC_NS, LAST_RESULT
    from concourse.bass_utils import run_bass_kernel_spmd

    sched = _build_schedule(q_ranges, k_ranges)
    key = _sched_key(sched)
    if key not in _NEFF_CACHE:
        _NEFF_CACHE[key] = _build_nc(sched)
    nc = _NEFF_CACHE[key]

    res = run_bass_kernel_spmd(
        nc, _shards(q, k, v, cos, sin), core_ids=list(range(N_CORES)),
        trace=PROFILE)
    LAST_RESULT = res
    LAST_EXEC_NS = getattr(res, "exec_time_ns", None)
    out = np.concatenate(
        [res.results[c]["out"].reshape(T, HPC, D) for c in range(N_CORES)],
        axis=1)
    return out.astype(np.float32, copy=False)


# revision 19
# speedup vs baseline: 2.5556x; 2.5556x over previous
"""Sparse (range-masked) GQA attention on 8 Trainium2 NeuronCores.

Strategy: tensor-parallel over heads. Core c owns q heads {2c, 2c+1} and kv
head c//2, so each core runs an independent single-core attention over the
full sequence — no collectives; the host concatenates the 8 head-slices.

Per-core kernel (Bass/Tile):
  1. DMA cos/sin/k/q/v shards into SBUF (in that order, so RoPE starts early).
  2. RoPE in fp32 on the vector engine (k first, then q heads), writing bf16.
  3. PE-transpose roped tiles into per-segment [d=128, 512] strips; the
     PSUM->SBUF copies ride the scalar engine, which is otherwise idle here.
  4. For each (head, 512-wide q segment, pair of active 128-wide k chunks):
       S^T[tk,tq] = KT_chunk.T @ QT_seg          (PE, bf16, PSUM fp32)
       P^T = Exp(scale*S^T - 4)                  (ACT, one [128,<=1024] instr)
       partial tiles only: P^T *= mask01         (masks pre-built in prologue)
       out[tq,0:129] += P^T_block.T @ [V | 1]    (PE; ones column => row sums)
  5. out = PV / max(rowsum, tiny); uncovered rows come out as exactly 0.

The tile schedule (skip / dense / partial+rectangles) is computed on the host
from q_ranges/k_ranges — scheduling metadata only; all arithmetic on q/k/v
runs on device. The compiled NEFF is cached per schedule signature.
"""

import math

import numpy as np

T, HQ, HKV, D = 2048, 16, 4, 128
N_CORES = 8
HPC = HQ // N_CORES          # q heads per core
SEG = 512                    # tq segment width (max moving free dim)
CK = 128                     # tk chunk (contraction tile)
NSEG = T // SEG
NCK = T // CK
NT = T // 128                # 128-row t-tiles
HALF = D // 2
SCALE = 1.0 / math.sqrt(D)
EXP_BIAS = -4.0              # constant shift; cancels in softmax normalization
SUM_EPS = 1e-30
MAX_PREBUILT_MASKS = 48      # SBUF budget cap; beyond this, build inline

PROFILE = False
LAST_EXEC_NS = None
LAST_RESULT = None

_NEFF_CACHE = {}


def _build_schedule(q_ranges, k_ranges):
    """Per (qseg, kchunk) tile: absent=skip, None=dense, list=mask rectangles.

    Rectangles are in device-tile coordinates: (klo, khi) along the partition
    (tk) axis, (qlo, qhi) along the free (tq) axis, clipped to the tile.
    """
    q_ranges = np.asarray(q_ranges, dtype=np.int64)
    k_ranges = np.asarray(k_ranges, dtype=np.int64)
    idx = np.arange(T)
    qm = (idx[None, :] >= q_ranges[:, :1]) & (idx[None, :] < q_ranges[:, 1:])
    km = (idx[None, :] >= k_ranges[:, :1]) & (idx[None, :] < k_ranges[:, 1:])
    mask = np.zeros((T, T), dtype=bool)
    for r in range(len(q_ranges)):
        if qm[r].any() and km[r].any():
            mask[np.ix_(qm[r], km[r])] = True
    sched = {}
    for s in range(NSEG):
        for c in range(NCK):
            sub = mask[s * SEG:(s + 1) * SEG, c * CK:(c + 1) * CK]
            if not sub.any():
                continue
            if sub.all():
                sched[(s, c)] = None
                continue
            rects = []
            for r in range(len(q_ranges)):
                qlo = max(int(q_ranges[r, 0]) - s * SEG, 0)
                qhi = min(int(q_ranges[r, 1]) - s * SEG, SEG)
                klo = max(int(k_ranges[r, 0]) - c * CK, 0)
                khi = min(int(k_ranges[r, 1]) - c * CK, CK)
                if qlo < qhi and klo < khi:
                    rects.append((klo, khi, qlo, qhi))
            sched[(s, c)] = rects
    return sched


def _sched_key(sched):
    return tuple(sorted(
        (k, -1) if v is None else (k, tuple(v)) for k, v in sched.items()
    ))


def _rect_mask(nc, AO, m, rect, first):
    """AND-chain of affine_select half-planes building a rect indicator."""
    klo, khi, qlo, qhi = rect
    if first:
        nc.gpsimd.memset(m, 1.0)
    if klo > 0:
        nc.gpsimd.affine_select(
            out=m, in_=m, compare_op=AO.is_ge, fill=0.0,
            base=-klo, pattern=[[0, SEG]], channel_multiplier=1)
    if khi < CK:
        nc.gpsimd.affine_select(
            out=m, in_=m, compare_op=AO.is_lt, fill=0.0,
            base=-khi, pattern=[[0, SEG]], channel_multiplier=1)
    if qlo > 0:
        nc.gpsimd.affine_select(
            out=m, in_=m, compare_op=AO.is_ge, fill=0.0,
            base=-qlo, pattern=[[1, SEG]], channel_multiplier=0)
    if qhi < SEG:
        nc.gpsimd.affine_select(
            out=m, in_=m, compare_op=AO.is_lt, fill=0.0,
            base=-qhi, pattern=[[1, SEG]], channel_multiplier=0)


def _build_mask_tile(nc, AO, pool, m01_tag, rects, bf16):
    m01 = pool.tile([128, SEG], bf16, tag=m01_tag, name=m01_tag)
    if len(rects) == 1:
        _rect_mask(nc, AO, m01[:], rects[0], first=True)
    else:
        acc = pool.tile([128, SEG], bf16, tag=m01_tag + "a", name=m01_tag + "a")
        _rect_mask(nc, AO, m01[:], rects[0], first=True)
        for rect in rects[1:]:
            _rect_mask(nc, AO, acc[:], rect, first=True)
            nc.vector.tensor_tensor(m01[:], m01[:], acc[:], AO.max)
    return m01


def _emit_body(nc, tc, pools, ext, sched, rep):
    from concourse import mybir

    f32 = mybir.dt.float32
    bf16 = mybir.dt.bfloat16
    AO = mybir.AluOpType
    AF = mybir.ActivationFunctionType
    big, ps_pool, pv_pool, pt_pool, out_pool, stat_pool = pools
    q_ext, k_ext, v_ext, cos_ext, sin_ext, out_ext = ext

    def btile(shape, dtype, tag):
        return big.tile(shape, dtype, tag=tag, name=f"{tag}_r{rep}")

    # -------- loads (k/cos/sin first, halved, so RoPE starts early) --------
    GT = NT // 2          # t-tiles per DMA half
    HT = GT * 128         # tokens per half
    cos_sb = btile([128, NT, HALF], f32, "cos_sb")
    sin_sb = btile([128, NT, HALF], f32, "sin_sb")
    k_sb = btile([128, NT, D], f32, "k_sb")
    q_sb = btile([128, NT, HPC, D], f32, "q_sb")
    v_sb = btile([128, NT, D], f32, "v_sb")
    for g in range(2):
        tt = slice(g * HT, (g + 1) * HT)
        nt = slice(g * GT, (g + 1) * GT)
        nc.sync.dma_start(
            k_sb[:, nt, :],
            k_ext[tt, :, :].rearrange("(n p) h d -> p n (h d)", p=128))
        nc.sync.dma_start(
            cos_sb[:, nt, :],
            cos_ext[tt, :].rearrange("(n p) d -> p n d", p=128))
        nc.sync.dma_start(
            sin_sb[:, nt, :],
            sin_ext[tt, :].rearrange("(n p) d -> p n d", p=128))
    for h in range(HPC):
        nc.sync.dma_start(
            q_sb[:, :, h, :],
            q_ext[:, h:h + 1, :].rearrange("(n p) h d -> p n (h d)", p=128))
    nc.sync.dma_start(
        v_sb[:], v_ext[:, :, :].rearrange("(n p) h d -> p n (h d)", p=128))

    # constants
    bias_sb = btile([128, 1], f32, "bias")
    nc.vector.memset(bias_sb[:], EXP_BIAS)
    zero_sb = btile([128, D], f32, "zero")
    nc.vector.memset(zero_sb[:], 0.0)

    # Partial tiles whose mask is one full-tk rectangle only restrict the tq
    # range: slice the exp to [qlo,qhi) and zero the rest — no mask needed.
    def _tq_window(rects):
        if len(rects) == 1 and rects[0][0] == 0 and rects[0][1] == CK:
            return rects[0][2], rects[0][3]
        return None

    # pre-built {0,1} masks for the remaining partial tiles
    partials = [key for key in sorted(sched)
                if sched[key] is not None and _tq_window(sched[key]) is None]
    mask_tiles = {}
    if len(partials) <= MAX_PREBUILT_MASKS:
        for mi, key in enumerate(partials):
            mask_tiles[key] = _build_mask_tile(
                nc, AO, big, f"msk{mi}", sched[key], bf16)

    # V with an appended ones column: [tk, (V | 1)] in bf16
    vaug = btile([128, NT, D + 1], bf16, "vaug")
    nc.gpsimd.memset(vaug[:], 1.0)
    nc.gpsimd.tensor_copy(vaug[:, :, 0:D], v_sb[:])

    # -------------- rope (k, then q heads) + DMA-transposes to strips ------
    # RoPE output goes to a DRAM scratch, then the DMA crossbar transpose
    # lands [d=128, 512] strips directly in SBUF — no PE/ACT/DVE involved.
    kts = [btile([128, SEG], bf16, f"kts{g}") for g in range(NSEG)]
    qts = [[btile([128, SEG], bf16, f"qts{h}_{g}") for g in range(NSEG)]
           for h in range(HPC)]
    t1 = btile([128, NT, HALF], f32, "t1")
    t2 = btile([128, NT, HALF], f32, "t2")
    t3 = btile([128, NT, HALF], f32, "t3")
    t4 = btile([128, NT, HALF], f32, "t4")
    plans = [(k_sb[:, :, :], kts)] + [
        (q_sb[:, :, h, :], qts[h]) for h in range(HPC)]
    for i, (x, strips) in enumerate(plans):
        dst = btile([128, NT, D], bf16, f"rope{i}")
        scr = nc.dram_tensor(f"scr{i}_r{rep}", [T, D], bf16)
        for g in range(2):
            nt = slice(g * GT, (g + 1) * GT)
            xlo, xhi = x[:, nt, 0:HALF], x[:, nt, HALF:D]
            cs, sn = cos_sb[:, nt, :], sin_sb[:, nt, :]
            ta, tb = t1[:, nt, :], t2[:, nt, :]
            tc, td = t3[:, nt, :], t4[:, nt, :]
            # the two xlo products ride gpsimd (~2x slower but otherwise idle)
            nc.vector.tensor_tensor(ta, xhi, sn, AO.mult)
            nc.gpsimd.tensor_tensor(tb, xlo, cs, AO.mult)
            nc.vector.tensor_tensor(dst[:, nt, 0:HALF], tb, ta, AO.subtract)
            nc.vector.tensor_tensor(tc, xhi, cs, AO.mult)
            nc.gpsimd.tensor_tensor(td, xlo, sn, AO.mult)
            nc.vector.tensor_tensor(dst[:, nt, HALF:D], tc, td, AO.add)
            nc.gpsimd.dma_start(
                scr[g * HT:(g + 1) * HT, :].rearrange("(n p) d -> p n d", p=128),
                dst[:, nt, :])
            for gg in range(2 * g, 2 * g + 2):
                nc.scalar.dma_start_transpose(
                    strips[gg][:], scr[gg * SEG:(gg + 1) * SEG, :])

    # ---------------- main attention loop ----------------
    for h in range(HPC):
        for s in range(NSEG):
            chunks = [c for c in range(NCK) if (s, c) in sched]
            if not chunks:
                for b in range(4):
                    q0 = s * SEG + b * 128
                    nc.sync.dma_start(
                        out_ext[q0:q0 + 128, h:h + 1, :], zero_sb[:])
                continue
            pairs = [chunks[i:i + 2] for i in range(0, len(chunks), 2)]
            pv = [pv_pool.tile([128, D + 1], f32, tag=f"pv{b}",
                               name=f"pv{b}_r{rep}_{h}_{s}", bufs=1)
                  for b in range(4)]
            for ip, pr in enumerate(pairs):
                w = len(pr) * SEG
                ps_s = ps_pool.tile([128, 2 * SEG], f32, tag="ps",
                                    name=f"ps_r{rep}_{h}_{s}_{ip}")
                for j, c in enumerate(pr):
                    nc.tensor.matmul(
                        ps_s[:, j * SEG:(j + 1) * SEG],
                        kts[c // 4][:, (c % 4) * 128:(c % 4 + 1) * 128],
                        qts[h][s][:], start=True, stop=True)
                p_t = pt_pool.tile([128, 2 * SEG], bf16, tag="pt",
                                   name=f"pt_r{rep}_{h}_{s}_{ip}")
                nc.scalar.activation(
                    p_t[:, 0:w], ps_s[:, 0:w], AF.Exp,
                    bias=bias_sb[:], scale=SCALE)
                for j, c in enumerate(pr):
                    rects = sched[(s, c)]
                    if rects is None:
                        continue
                    if (s, c) in mask_tiles:
                        m01 = mask_tiles[(s, c)]
                    else:
                        m01 = _build_mask_tile(nc, AO, pt_pool, "m01", rects, bf16)
                    sl = slice(j * SEG, (j + 1) * SEG)
                    nc.vector.tensor_tensor(p_t[:, sl], p_t[:, sl], m01[:], AO.mult)
                first, last = ip == 0, ip == len(pairs) - 1
                for j, c in enumerate(pr):
                    for b in range(4):
                        nc.tensor.matmul(
                            pv[b][:],
                            p_t[:, j * SEG + b * 128:j * SEG + (b + 1) * 128],
                            vaug[:, c, :],
                            start=(first and j == 0),
                            stop=(last and j == len(pr) - 1))
            for b in range(4):
                src = pv[b]
                sums = stat_pool.tile([128, 1], f32, tag="sums",
                                      name=f"sums_r{rep}_{h}_{s}_{b}")
                nc.vector.tensor_scalar_max(sums[:], src[:, D:D + 1], SUM_EPS)
                rec = stat_pool.tile([128, 1], f32, tag="rec",
                                     name=f"rec_r{rep}_{h}_{s}_{b}")
                nc.vector.reciprocal(rec[:], sums[:])
                o_sb = out_pool.tile([128, D], f32, tag="o",
                                     name=f"o_r{rep}_{h}_{s}_{b}")
                nc.vector.tensor_scalar_mul(o_sb[:], src[:, 0:D], rec[:])
                q0 = s * SEG + b * 128
                nc.sync.dma_start(out_ext[q0:q0 + 128, h:h + 1, :], o_sb[:])


def _build_nc(sched, reps=1):
    from contextlib import ExitStack

    from concourse import bacc, mybir, tile

    f32 = mybir.dt.float32

    nc = bacc.Bacc(None, target_bir_lowering=False)
    q_ext = nc.declare_dram_parameter("q", [T, HPC, D], f32, isOutput=False)
    k_ext = nc.declare_dram_parameter("k", [T, 1, D], f32, isOutput=False)
    v_ext = nc.declare_dram_parameter("v", [T, 1, D], f32, isOutput=False)
    cos_ext = nc.declare_dram_parameter("cos", [T, HALF], f32, isOutput=False)
    sin_ext = nc.declare_dram_parameter("sin", [T, HALF], f32, isOutput=False)
    out_ext = nc.declare_dram_parameter("out", [T, HPC, D], f32, isOutput=True)
    ext = (q_ext, k_ext, v_ext, cos_ext, sin_ext, out_ext)

    with tile.TileContext(nc) as tc, ExitStack() as ctx:
        big = ctx.enter_context(tc.tile_pool(name="big", bufs=1))
        ps_pool = ctx.enter_context(
            tc.tile_pool(name="psum", bufs=2, space="PSUM"))
        pv_pool = ctx.enter_context(
            tc.tile_pool(name="pvp", bufs=1, space="PSUM"))
        pt_pool = ctx.enter_context(tc.tile_pool(name="ptsb", bufs=3))
        out_pool = ctx.enter_context(tc.tile_pool(name="outp", bufs=4))
        stat_pool = ctx.enter_context(tc.tile_pool(name="stat", bufs=8))
        pools = (big, ps_pool, pv_pool, pt_pool, out_pool, stat_pool)
        for rep in range(reps):
            _emit_body(nc, tc, pools, ext, sched, rep)
    nc.compile()
    return nc


def _shards(q, k, v, cos, sin):
    in_maps = []
    for c in range(N_CORES):
        kv = c // 2
        in_maps.append({
            "q": np.ascontiguousarray(q[:, 2 * c:2 * c + 2, :], dtype=np.float32),
            "k": np.ascontiguousarray(k[:, kv:kv + 1, :], dtype=np.float32),
            "v": np.ascontiguousarray(v[:, kv:kv + 1, :], dtype=np.float32),
            "cos": np.ascontiguousarray(cos, dtype=np.float32),
            "sin": np.ascontiguousarray(sin, dtype=np.float32),
        })
    return in_maps


def kernel(q, k, v, cos, sin, q_ranges, k_ranges):
    global LAST_EXEC_NS, LAST_RESULT
    from concourse.bass_utils import run_bass_kernel_spmd

    sched = _build_schedule(q_ranges, k_ranges)
    key = _sched_key(sched)
    if key not in _NEFF_CACHE:
        _NEFF_CACHE[key] = _build_nc(sched)
    nc = _NEFF_CACHE[key]

    res = run_bass_kernel_spmd(
        nc, _shards(q, k, v, cos, sin), core_ids=list(range(N_CORES)),
        trace=PROFILE)
    LAST_RESULT = res
    LAST_EXEC_NS = getattr(res, "exec_time_ns", None)
    out = np.concatenate(
        [res.results[c]["out"].reshape(T, HPC, D) for c in range(N_CORES)],
        axis=1)
    return out.astype(np.float32, copy=False)


# revision 21
# speedup vs baseline: 3.4255x; 1.3404x over previous
"""Sparse (range-masked) GQA attention on 8 Trainium2 NeuronCores.

Strategy: tensor-parallel over heads. Core c owns q heads {2c, 2c+1} and kv
head c//2, so each core runs an independent single-core attention over the
full sequence — no collectives; the host concatenates the 8 head-slices.

Per-core kernel (Bass/Tile):
  1. DMA cos/sin/k/q/v shards into SBUF (in that order, so RoPE starts early).
  2. RoPE in fp32 on the vector engine (k first, then q heads), writing bf16.
  3. PE-transpose roped tiles into per-segment [d=128, 512] strips; the
     PSUM->SBUF copies ride the scalar engine, which is otherwise idle here.
  4. For each (head, 512-wide q segment, pair of active 128-wide k chunks):
       S^T[tk,tq] = KT_chunk.T @ QT_seg          (PE, bf16, PSUM fp32)
       P^T = Exp(scale*S^T - 4)                  (ACT, one [128,<=1024] instr)
       partial tiles only: P^T *= mask01         (masks pre-built in prologue)
       out[tq,0:129] += P^T_block.T @ [V | 1]    (PE; ones column => row sums)
  5. out = PV / max(rowsum, tiny); uncovered rows come out as exactly 0.

The tile schedule (skip / dense / partial+rectangles) is computed on the host
from q_ranges/k_ranges — scheduling metadata only; all arithmetic on q/k/v
runs on device. The compiled NEFF is cached per schedule signature.
"""

import math

import numpy as np

T, HQ, HKV, D = 2048, 16, 4, 128
N_CORES = 8
HPC = HQ // N_CORES          # q heads per core
SEG = 512                    # tq segment width (max moving free dim)
CK = 128                     # tk chunk (contraction tile)
NSEG = T // SEG
NCK = T // CK
NT = T // 128                # 128-row t-tiles
HALF = D // 2
SCALE = 1.0 / math.sqrt(D)
EXP_BIAS = -4.0              # constant shift; cancels in softmax normalization
SUM_EPS = 1e-30
MAX_PREBUILT_MASKS = 48      # SBUF budget cap; beyond this, build inline

PROFILE = False
LAST_EXEC_NS = None
LAST_RESULT = None

_NEFF_CACHE = {}


def _build_schedule(q_ranges, k_ranges):
    """Per (qseg, kchunk) tile: absent=skip, None=dense, list=mask rectangles.

    Rectangles are in device-tile coordinates: (klo, khi) along the partition
    (tk) axis, (qlo, qhi) along the free (tq) axis, clipped to the tile.
    """
    q_ranges = np.asarray(q_ranges, dtype=np.int64)
    k_ranges = np.asarray(k_ranges, dtype=np.int64)
    idx = np.arange(T)
    qm = (idx[None, :] >= q_ranges[:, :1]) & (idx[None, :] < q_ranges[:, 1:])
    km = (idx[None, :] >= k_ranges[:, :1]) & (idx[None, :] < k_ranges[:, 1:])
    mask = np.zeros((T, T), dtype=bool)
    for r in range(len(q_ranges)):
        if qm[r].any() and km[r].any():
            mask[np.ix_(qm[r], km[r])] = True
    sched = {}
    for s in range(NSEG):
        for c in range(NCK):
            sub = mask[s * SEG:(s + 1) * SEG, c * CK:(c + 1) * CK]
            if not sub.any():
                continue
            if sub.all():
                sched[(s, c)] = None
                continue
            rects = []
            for r in range(len(q_ranges)):
                qlo = max(int(q_ranges[r, 0]) - s * SEG, 0)
                qhi = min(int(q_ranges[r, 1]) - s * SEG, SEG)
                klo = max(int(k_ranges[r, 0]) - c * CK, 0)
                khi = min(int(k_ranges[r, 1]) - c * CK, CK)
                if qlo < qhi and klo < khi:
                    rects.append((klo, khi, qlo, qhi))
            sched[(s, c)] = rects
    return sched


def _sched_key(sched):
    return tuple(sorted(
        (k, -1) if v is None else (k, tuple(v)) for k, v in sched.items()
    ))


def _rect_mask(nc, AO, m, rect, first):
    """AND-chain of affine_select half-planes building a rect indicator."""
    klo, khi, qlo, qhi = rect
    if first:
        nc.gpsimd.memset(m, 1.0)
    if klo > 0:
        nc.gpsimd.affine_select(
            out=m, in_=m, compare_op=AO.is_ge, fill=0.0,
            base=-klo, pattern=[[0, SEG]], channel_multiplier=1)
    if khi < CK:
        nc.gpsimd.affine_select(
            out=m, in_=m, compare_op=AO.is_lt, fill=0.0,
            base=-khi, pattern=[[0, SEG]], channel_multiplier=1)
    if qlo > 0:
        nc.gpsimd.affine_select(
            out=m, in_=m, compare_op=AO.is_ge, fill=0.0,
            base=-qlo, pattern=[[1, SEG]], channel_multiplier=0)
    if qhi < SEG:
        nc.gpsimd.affine_select(
            out=m, in_=m, compare_op=AO.is_lt, fill=0.0,
            base=-qhi, pattern=[[1, SEG]], channel_multiplier=0)


def _build_mask_tile(nc, AO, pool, m01_tag, rects, bf16):
    m01 = pool.tile([128, SEG], bf16, tag=m01_tag, name=m01_tag)
    if len(rects) == 1:
        _rect_mask(nc, AO, m01[:], rects[0], first=True)
    else:
        acc = pool.tile([128, SEG], bf16, tag=m01_tag + "a", name=m01_tag + "a")
        _rect_mask(nc, AO, m01[:], rects[0], first=True)
        for rect in rects[1:]:
            _rect_mask(nc, AO, acc[:], rect, first=True)
            nc.vector.tensor_tensor(m01[:], m01[:], acc[:], AO.max)
    return m01


def _emit_body(nc, tc, pools, ext, sched, rep):
    from concourse import mybir

    f32 = mybir.dt.float32
    bf16 = mybir.dt.bfloat16
    AO = mybir.AluOpType
    AF = mybir.ActivationFunctionType
    big, ps_pool, pv_pool, pt_pool, out_pool, stat_pool = pools
    q_ext, k_ext, v_ext, cos_ext, sin_ext, out_ext = ext

    def btile(shape, dtype, tag):
        return big.tile(shape, dtype, tag=tag, name=f"{tag}_r{rep}")

    # -------- loads (k/cos/sin first, halved, so RoPE starts early) --------
    GT = NT // 2          # t-tiles per DMA half
    HT = GT * 128         # tokens per half
    cos_sb = btile([128, NT, HALF], f32, "cos_sb")
    sin_sb = btile([128, NT, HALF], f32, "sin_sb")
    k_sb = btile([128, NT, D], f32, "k_sb")
    q_sb = btile([128, NT, HPC, D], f32, "q_sb")
    v_sb = btile([128, NT, D], f32, "v_sb")
    def load_half(g):
        tt = slice(g * HT, (g + 1) * HT)
        nt = slice(g * GT, (g + 1) * GT)
        nc.sync.dma_start(
            k_sb[:, nt, :],
            k_ext[tt, :, :].rearrange("(n p) h d -> p n (h d)", p=128))
        nc.sync.dma_start(
            cos_sb[:, nt, :],
            cos_ext[tt, :].rearrange("(n p) d -> p n d", p=128))
        nc.sync.dma_start(
            sin_sb[:, nt, :],
            sin_ext[tt, :].rearrange("(n p) d -> p n d", p=128))

    def load_q(h, g):
        tt = slice(g * HT, (g + 1) * HT)
        nt = slice(g * GT, (g + 1) * GT)
        nc.sync.dma_start(
            q_sb[:, nt, h, :],
            q_ext[tt, h:h + 1, :].rearrange("(n p) h d -> p n (h d)", p=128))

    load_half(0)
    load_q(0, 0)
    load_half(1)
    load_q(0, 1)
    load_q(1, 0)
    load_q(1, 1)
    nc.sync.dma_start(
        v_sb[:], v_ext[:, :, :].rearrange("(n p) h d -> p n (h d)", p=128))

    # constants
    bias_sb = btile([128, 1], f32, "bias")
    nc.vector.memset(bias_sb[:], EXP_BIAS)
    zero_sb = btile([128, D], f32, "zero")
    nc.vector.memset(zero_sb[:], 0.0)

    # Partial tiles whose mask is one full-tk rectangle only restrict the tq
    # range: slice the exp to [qlo,qhi) and zero the rest — no mask needed.
    def _tq_window(rects):
        if len(rects) == 1 and rects[0][0] == 0 and rects[0][1] == CK:
            return rects[0][2], rects[0][3]
        return None

    # pre-built {0,1} masks for the remaining partial tiles
    partials = [key for key in sorted(sched)
                if sched[key] is not None and _tq_window(sched[key]) is None]
    mask_tiles = {}
    if len(partials) <= MAX_PREBUILT_MASKS:
        for mi, key in enumerate(partials):
            mask_tiles[key] = _build_mask_tile(
                nc, AO, big, f"msk{mi}", sched[key], bf16)

    # V with an appended ones column: [tk, (V | 1)] in bf16
    vaug = btile([128, NT, D + 1], bf16, "vaug")
    nc.gpsimd.memset(vaug[:], 1.0)
    nc.gpsimd.tensor_copy(vaug[:, :, 0:D], v_sb[:])

    # -------------- rope (k, then q heads) + DMA-transposes to strips ------
    # RoPE output goes to a DRAM scratch, then the DMA crossbar transpose
    # lands [d=128, 512] strips directly in SBUF — no PE/ACT/DVE involved.
    kts = [btile([128, SEG], bf16, f"kts{g}") for g in range(NSEG)]
    qts = [[btile([128, SEG], bf16, f"qts{h}_{g}") for g in range(NSEG)]
           for h in range(HPC)]
    t1 = btile([128, NT, HALF], f32, "t1")
    t2 = btile([128, NT, HALF], f32, "t2")
    t3 = btile([128, NT, HALF], f32, "t3")
    t4 = btile([128, NT, HALF], f32, "t4")
    plans = [(k_sb[:, :, :], kts)] + [
        (q_sb[:, :, h, :], qts[h]) for h in range(HPC)]
    for i, (x, strips) in enumerate(plans):
        dst = btile([128, NT, D], bf16, f"rope{i}")
        scr = nc.dram_tensor(f"scr{i}_r{rep}", [T, D], bf16)
        for g in range(2):
            nt = slice(g * GT, (g + 1) * GT)
            xlo, xhi = x[:, nt, 0:HALF], x[:, nt, HALF:D]
            cs, sn = cos_sb[:, nt, :], sin_sb[:, nt, :]
            ta, tb = t1[:, nt, :], t2[:, nt, :]
            tc, td = t3[:, nt, :], t4[:, nt, :]
            # the two xlo products ride gpsimd (~2x slower but otherwise idle)
            nc.vector.tensor_tensor(ta, xhi, sn, AO.mult)
            nc.gpsimd.tensor_tensor(tb, xlo, cs, AO.mult)
            nc.vector.tensor_tensor(dst[:, nt, 0:HALF], tb, ta, AO.subtract)
            nc.vector.tensor_tensor(tc, xhi, cs, AO.mult)
            nc.gpsimd.tensor_tensor(td, xlo, sn, AO.mult)
            nc.vector.tensor_tensor(dst[:, nt, HALF:D], tc, td, AO.add)
            nc.gpsimd.dma_start(
                scr[g * HT:(g + 1) * HT, :].rearrange("(n p) d -> p n d", p=128),
                dst[:, nt, :])
            for gg in range(2 * g, 2 * g + 2):
                nc.scalar.dma_start_transpose(
                    strips[gg][:], scr[gg * SEG:(gg + 1) * SEG, :])

    # ---------------- main attention loop ----------------
    for h in range(HPC):
        for s in range(NSEG):
            chunks = [c for c in range(NCK) if (s, c) in sched]
            if not chunks:
                for b in range(4):
                    q0 = s * SEG + b * 128
                    nc.sync.dma_start(
                        out_ext[q0:q0 + 128, h:h + 1, :], zero_sb[:])
                continue
            # active tq window per chunk; masked partials use the full window
            wins = {}
            for c in chunks:
                rects = sched[(s, c)]
                win = None if rects is None else _tq_window(rects)
                wins[c] = win if win is not None else (0, SEG)

            def overlap(c, b):
                return wins[c][0] < (b + 1) * 128 and b * 128 < wins[c][1]

            totals = {b: sum(1 for c in chunks if overlap(c, b))
                      for b in range(4)}
            counts = {b: 0 for b in range(4)}
            pairs = [chunks[i:i + 2] for i in range(0, len(chunks), 2)]
            pv = [pv_pool.tile([128, D + 1], f32, tag=f"pv{b}",
                               name=f"pv{b}_r{rep}_{h}_{s}", bufs=1)
                  for b in range(4)]
            for ip, pr in enumerate(pairs):
                ps_s = ps_pool.tile([128, 2 * SEG], f32, tag="ps",
                                    name=f"ps_r{rep}_{h}_{s}_{ip}")
                for j, c in enumerate(pr):
                    qlo, qhi = wins[c]
                    base = j * SEG
                    nc.tensor.matmul(
                        ps_s[:, base + qlo:base + qhi],
                        kts[c // 4][:, (c % 4) * 128:(c % 4 + 1) * 128],
                        qts[h][s][:, qlo:qhi], start=True, stop=True)
                p_t = pt_pool.tile([128, 2 * SEG], bf16, tag="pt",
                                   name=f"pt_r{rep}_{h}_{s}_{ip}")
                merged = []
                for j, c in enumerate(pr):
                    qlo, qhi = wins[c]
                    base = j * SEG
                    if qlo > 0:
                        nc.gpsimd.memset(p_t[:, base:base + qlo], 0.0)
                    if qhi < SEG:
                        nc.gpsimd.memset(p_t[:, base + qhi:base + SEG], 0.0)
                    if merged and merged[-1][1] == base + qlo:
                        merged[-1][1] = base + qhi
                    else:
                        merged.append([base + qlo, base + qhi])
                for a, b_ in merged:
                    nc.scalar.activation(
                        p_t[:, a:b_], ps_s[:, a:b_], AF.Exp,
                        bias=bias_sb[:], scale=SCALE)
                for j, c in enumerate(pr):
                    rects = sched[(s, c)]
                    if rects is None or _tq_window(rects) is not None:
                        continue
                    if (s, c) in mask_tiles:
                        m01 = mask_tiles[(s, c)]
                    else:
                        m01 = _build_mask_tile(nc, AO, pt_pool, "m01", rects, bf16)
                    sl = slice(j * SEG, (j + 1) * SEG)
                    nc.vector.tensor_tensor(p_t[:, sl], p_t[:, sl], m01[:], AO.mult)
                for j, c in enumerate(pr):
                    for b in range(4):
                        if not overlap(c, b):
                            continue
                        st = counts[b] == 0
                        sp = counts[b] == totals[b] - 1
                        counts[b] += 1
                        nc.tensor.matmul(
                            pv[b][:],
                            p_t[:, j * SEG + b * 128:j * SEG + (b + 1) * 128],
                            vaug[:, c, :], start=st, stop=sp)
            for b in range(4):
                if totals[b] == 0:
                    q0 = s * SEG + b * 128
                    nc.sync.dma_start(
                        out_ext[q0:q0 + 128, h:h + 1, :], zero_sb[:])
                    continue
                src = pv[b]
                sums = stat_pool.tile([128, 1], f32, tag="sums",
                                      name=f"sums_r{rep}_{h}_{s}_{b}")
                nc.vector.tensor_scalar_max(sums[:], src[:, D:D + 1], SUM_EPS)
                rec = stat_pool.tile([128, 1], f32, tag="rec",
                                     name=f"rec_r{rep}_{h}_{s}_{b}")
                nc.vector.reciprocal(rec[:], sums[:])
                o_sb = out_pool.tile([128, D], f32, tag="o",
                                     name=f"o_r{rep}_{h}_{s}_{b}")
                nc.vector.tensor_scalar_mul(o_sb[:], src[:, 0:D], rec[:])
                q0 = s * SEG + b * 128
                nc.sync.dma_start(out_ext[q0:q0 + 128, h:h + 1, :], o_sb[:])


def _build_nc(sched, reps=1):
    from contextlib import ExitStack

    from concourse import bacc, mybir, tile

    f32 = mybir.dt.float32

    nc = bacc.Bacc(None, target_bir_lowering=False)
    q_ext = nc.declare_dram_parameter("q", [T, HPC, D], f32, isOutput=False)
    k_ext = nc.declare_dram_parameter("k", [T, 1, D], f32, isOutput=False)
    v_ext = nc.declare_dram_parameter("v", [T, 1, D], f32, isOutput=False)
    cos_ext = nc.declare_dram_parameter("cos", [T, HALF], f32, isOutput=False)
    sin_ext = nc.declare_dram_parameter("sin", [T, HALF], f32, isOutput=False)
    out_ext = nc.declare_dram_parameter("out", [T, HPC, D], f32, isOutput=True)
    ext = (q_ext, k_ext, v_ext, cos_ext, sin_ext, out_ext)

    with tile.TileContext(nc) as tc, ExitStack() as ctx:
        big = ctx.enter_context(tc.tile_pool(name="big", bufs=1))
        ps_pool = ctx.enter_context(
            tc.tile_pool(name="psum", bufs=2, space="PSUM"))
        pv_pool = ctx.enter_context(
            tc.tile_pool(name="pvp", bufs=1, space="PSUM"))
        pt_pool = ctx.enter_context(tc.tile_pool(name="ptsb", bufs=3))
        out_pool = ctx.enter_context(tc.tile_pool(name="outp", bufs=4))
        stat_pool = ctx.enter_context(tc.tile_pool(name="stat", bufs=8))
        pools = (big, ps_pool, pv_pool, pt_pool, out_pool, stat_pool)
        for rep in range(reps):
            _emit_body(nc, tc, pools, ext, sched, rep)
    nc.compile()
    return nc


def _shards(q, k, v, cos, sin):
    in_maps = []
    for c in range(N_CORES):
        kv = c // 2
        in_maps.append({
            "q": np.ascontiguousarray(q[:, 2 * c:2 * c + 2, :], dtype=np.float32),
            "k": np.ascontiguousarray(k[:, kv:kv + 1, :], dtype=np.float32),
            "v": np.ascontiguousarray(v[:, kv:kv + 1, :], dtype=np.float32),
            "cos": np.ascontiguousarray(cos, dtype=np.float32),
            "sin": np.ascontiguousarray(sin, dtype=np.float32),
        })
    return in_maps


def kernel(q, k, v, cos, sin, q_ranges, k_ranges):
    global LAST_EXEC_NS, LAST_RESULT
    from concourse.bass_utils import run_bass_kernel_spmd

    sched = _build_schedule(q_ranges, k_ranges)
    key = _sched_key(sched)
    if key not in _NEFF_CACHE:
        _NEFF_CACHE[key] = _build_nc(sched)
    nc = _NEFF_CACHE[key]

    res = run_bass_kernel_spmd(
        nc, _shards(q, k, v, cos, sin), core_ids=list(range(N_CORES)),
        trace=PROFILE)
    LAST_RESULT = res
    LAST_EXEC_NS = getattr(res, "exec_time_ns", None)
    out = np.concatenate(
        [res.results[c]["out"].reshape(T, HPC, D) for c in range(N_CORES)],
        axis=1)
    return out.astype(np.float32, copy=False)
